# revision 14
# baseline (speedup 1.0000x reference)
"""Trainium2 Bass kernel for a 2-layer GraphSAGE (segment-mean aggregation).

Single fused SPMD program on 8 cores. Nodes are sharded contiguously by id;
edges partitioned by destination so each core's scatter-mean is local. The
halo exchanges are ON-DEVICE AllGather collectives: x shards are gathered
into a Shared-DRAM table before layer 1, and the layer-1 node features are
gathered into a second Shared table before layer 2 — no host round-trip.

Per bin (<=32 consecutive nodes, <=512 edges) the device gathers table rows
(indirect DMA, one 128-row gather per edge tile), builds a recip-scaled
one-hot on DVE, and a TensorE matmul accumulates feature-major segment means
into PSUM. Per 4 bins, two more matmuls apply W_l/W_r and an epilogue adds
bias (+relu between layers). The final output travels over the wire as fp16
(tolerance is 2e-2; fp16 rounding is ~1e-3 of max).

Host side: the axon tunnel moves ~30 MB/s, so everything re-usable is staged
on device once and cached by content digest — x, gather metadata, weights,
and the jitted shard_map callable. Steady-state per-call traffic is just the
fp16 output fetch.
"""

import sys
import zlib
from contextlib import ExitStack
from functools import partial

import numpy as np

try:
    import concourse.bass as bass
except ImportError:  # pragma: no cover
    sys.path.insert(0, "/opt/trn_rl_repo")
    import concourse.bass as bass

import jax
import jax.numpy as jnp
from jax.sharding import Mesh, NamedSharding, PartitionSpec

from jax.experimental.shard_map import shard_map

import concourse.bacc as bacc
import concourse.mybir as mybir
import concourse.tile as tile
from concourse.bass2jax import (
    _bass_exec_p,
    install_neuronx_cc_hook,
    partition_id_tensor,
)
from concourse.masks import make_identity

N = 50000
E = 800000
D = 128
NC = 8
NSH = N // NC
T = 4
SLOTS_PER_BIN = T * 128
NPB = 32
GROUP = 4
BIN_ROUND = 8
OWN_CB = 4

F32 = mybir.dt.float32
F16 = mybir.dt.float16
I32 = mybir.dt.int32
I8 = mybir.dt.int8
QMAX = 126.5


def build_metadata(edge_index, n_nodes=N, n_cores=NC):
    src = np.asarray(edge_index[0], dtype=np.int64)
    dst = np.asarray(edge_index[1], dtype=np.int64)
    nsh = n_nodes // n_cores
    deg = np.bincount(dst, minlength=n_nodes)
    assert deg.max() <= SLOTS_PER_BIN
    recip = np.zeros(n_nodes, np.float32)
    nz = deg > 0
    recip[nz] = (1.0 / deg[nz]).astype(np.float32)

    order = np.argsort(dst, kind="stable")
    src_s = src[order]
    indptr = np.zeros(n_nodes + 1, np.int64)
    indptr[1:] = np.cumsum(deg)

    core_bins = []
    for c in range(n_cores):
        lo, hi = c * nsh, (c + 1) * nsh
        bins = []
        i = lo
        while i < hi:
            start = i
            s = 0
            while i < hi and (i - start) < NPB and s + deg[i] <= SLOTS_PER_BIN:
                s += deg[i]
                i += 1
            bins.append((start, i - start))
        core_bins.append(bins)

    B = max(len(b) for b in core_bins)
    B = -(-B // BIN_ROUND) * BIN_ROUND
    NSLOT = B * NPB
    OWN_C = NSLOT // 128
    NBATCH = B // BIN_ROUND
    OWN_CHUNKS = -(-OWN_C // OWN_CB)

    C = B * T
    gidx1 = np.zeros((n_cores, 128, C), np.int32)
    gidx2 = np.zeros((n_cores, 128, C), np.int32)
    seg = np.zeros((n_cores, 128, C), np.float32)
    rcp = np.zeros((n_cores, 128, C), np.float32)
    ownidx = np.zeros((n_cores, 128, OWN_C), np.int32)
    node_pos = np.full(n_nodes, -1, np.int64)

    for c in range(n_cores):
        for b, (nlo, nn) in enumerate(core_bins[c]):
            base = b * NPB
            nodes = np.arange(nlo, nlo + nn)
            slots = base + np.arange(nn)
            node_pos[nodes] = c * NSLOT + slots
            # local ids: the layer-1 own-feature gather reads this core's
            # x shard, not the global table
            ownidx[c, slots % 128, slots // 128] = nodes - c * nsh
            degs = deg[nodes]
            ne = int(degs.sum())
            if ne == 0:
                continue
            s = np.arange(ne)
            q = np.repeat(np.arange(nn), degs)
            e0 = indptr[nlo]
            t_, p_ = s // 128, s % 128
            col = b * T + t_
            gidx1[c, p_, col] = src_s[e0:e0 + ne]
            seg[c, p_, col] = q
            rcp[c, p_, col] = np.repeat(recip[nodes], degs)

    assert np.all(node_pos >= 0)
    for c in range(n_cores):
        g2 = node_pos[gidx1[c]].astype(np.int32)
        g2[rcp[c] == 0.0] = 0
        gidx2[c] = g2

    def batched(a, w):
        nb = a.shape[-1] // w
        return np.ascontiguousarray(
            a.reshape(a.shape[0], 128, nb, w).transpose(0, 2, 1, 3))

    bw = BIN_ROUND * T
    md = dict(B=B, C=C, NSLOT=NSLOT, OWN_C=OWN_C, NBATCH=NBATCH,
              OWN_CHUNKS=OWN_CHUNKS, node_pos=node_pos,
              g1=batched(gidx1, bw), g2=batched(gidx2, bw),
              sg=batched(seg, bw), rc=batched(rcp, bw))
    pad = OWN_CHUNKS * OWN_CB - OWN_C
    if pad:
        ownidx = np.concatenate(
            [ownidx, np.zeros((n_cores, 128, pad), np.int32)], axis=-1)
    md["own"] = batched(ownidx, OWN_CB)
    md["iota"] = np.tile(np.arange(NPB, dtype=np.float32), (128, 1))
    return md


def build_program(B, n_nodes=N, n_cores=NC):
    NSLOT = B * NPB
    OWN_C = NSLOT // 128
    NBATCH = B // BIN_ROUND
    OWN_CHUNKS = -(-OWN_C // OWN_CB)
    NGROUP = B // GROUP
    bw = BIN_ROUND * T
    RG = [list(range(n_cores))]

    nc = bacc.Bacc("TRN2", target_bir_lowering=False, debug=False,
                   num_devices=n_cores)

    xs_ext = nc.dram_tensor("xs", [NSH, D], F32, kind="ExternalInput")
    g1_ext = nc.dram_tensor("g1", [NBATCH, 128, bw], I32, kind="ExternalInput")
    g2_ext = nc.dram_tensor("g2", [NBATCH, 128, bw], I32, kind="ExternalInput")
    sg_ext = nc.dram_tensor("sg", [NBATCH, 128, bw], F32, kind="ExternalInput")
    rc_ext = nc.dram_tensor("rc", [NBATCH, 128, bw], F32, kind="ExternalInput")
    own_ext = nc.dram_tensor("own", [OWN_CHUNKS, 128, OWN_CB], I32,
                             kind="ExternalInput")
    iota_ext = nc.dram_tensor("iota", [128, NPB], F32, kind="ExternalInput")
    w_ext = {k: nc.dram_tensor(k, [D, D], F32, kind="ExternalInput")
             for k in ("W1l", "W1r", "W2l", "W2r")}
    b_ext = {k: nc.dram_tensor(k, [D, 1], F32, kind="ExternalInput")
             for k in ("b1", "b2")}
    # one fused output: 128 int8 payload bytes + 4 bytes (f32 scale) per
    # (group, feature) row — a single fetch round-trip over the slow tunnel
    out_ext = nc.dram_tensor("outT", [NGROUP, D, GROUP * NPB + 4], I8,
                             kind="ExternalOutput")

    xb = nc.dram_tensor("xb", [NSH, D], F32)
    xfull = nc.dram_tensor("xfull", [n_nodes, D], F32, addr_space="Shared")
    hb = nc.dram_tensor("hb", [NSLOT, D], F32)
    hfull = nc.dram_tensor("hfull", [n_cores * NSLOT, D], F32,
                           addr_space="Shared")

    with tile.TileContext(nc) as tc, ExitStack() as ctx:
        const = ctx.enter_context(tc.tile_pool(name="const", bufs=1))
        gpool = ctx.enter_context(tc.tile_pool(name="gather", bufs=3))
        mpool = ctx.enter_context(tc.tile_pool(name="meta", bufs=4))
        ohpool = ctx.enter_context(tc.tile_pool(name="oh", bufs=4))
        stpool = ctx.enter_context(tc.tile_pool(name="stage", bufs=4))
        pseg = ctx.enter_context(tc.tile_pool(name="pseg", bufs=2, space="PSUM"))
        pw = ctx.enter_context(tc.tile_pool(name="pw", bufs=2, space="PSUM"))
        pt = ctx.enter_context(tc.tile_pool(name="pt", bufs=2, space="PSUM"))

        # kick off the x all-gather first; layer-1 edge gathers wait on it,
        # the own-feature path below does not
        nc.gpsimd.dma_start(xb[:, :], xs_ext[:, :])
        nc.gpsimd.collective_compute(
            "AllGather", mybir.AluOpType.bypass, replica_groups=RG,
            ins=[xb[:, :]], outs=[xfull[:, :]])

        W = {}
        for k in ("W1l", "W1r", "W2l", "W2r"):
            W[k] = const.tile([D, D], F32, name=k)
            nc.sync.dma_start(W[k][:], w_ext[k][:, :])
        bias = {}
        for k in ("b1", "b2"):
            bias[k] = const.tile([D, 1], F32, name=k)
            nc.sync.dma_start(bias[k][:], b_ext[k][:, :])
        iota_sb = const.tile([128, NPB], F32, name="iota_sb")
        nc.sync.dma_start(iota_sb[:], iota_ext[:, :])
        ident = const.tile([128, 128], F32, name="ident")
        make_identity(nc, ident[:])

        def iota_rep(k):
            ap = iota_sb[:, :]
            return bass.AP(ap.tensor, ap.offset,
                           [[NPB, 128], [0, k], [1, NPB]])

        def own_transpose(ownT, layer):
            """ownT = (own-node features)^T, feature-major [128, NSLOT]."""
            if layer == 1:
                for chk in range(OWN_CHUNKS):
                    oi = mpool.tile([128, OWN_CB], I32, tag="oi", name="oi")
                    nc.sync.dma_start(oi[:], own_ext[chk])
                    ob = gpool.tile([128, OWN_CB * 128], F32, tag="ob",
                                    name="ob")
                    for j in range(OWN_CB):
                        nc.gpsimd.indirect_dma_start(
                            out=ob[:, j * 128:(j + 1) * 128], out_offset=None,
                            in_=xs_ext[:, :],
                            in_offset=bass.IndirectOffsetOnAxis(
                                ap=oi[:, j:j + 1], axis=0))
                    for j in range(OWN_CB):
                        col = chk * OWN_CB + j
                        if col >= OWN_C:
                            break
                        tp = pt.tile([128, 128], F32, tag="tp", name="tp")
                        nc.tensor.transpose(
                            tp[:], ob[:, j * 128:(j + 1) * 128], ident[:])
                        nc.vector.tensor_copy(
                            ownT[:, col * 128:(col + 1) * 128], tp[:])
            else:
                for g in range(OWN_C):
                    ho = gpool.tile([128, 128], F32, tag="ho", name="ho")
                    nc.sync.dma_start(ho[:], hb[g * 128:(g + 1) * 128, :])
                    tp = pt.tile([128, 128], F32, tag="tp", name="tp")
                    nc.tensor.transpose(tp[:], ho[:], ident[:])
                    nc.vector.tensor_copy(ownT[:, g * 128:(g + 1) * 128],
                                          tp[:])

        def layer(layer_no, tbl, g_ext, Wl, Wr, bs, ownT):
            for eb in range(NBATCH):
                gi = mpool.tile([128, bw], I32, tag="gi", name="gi")
                nc.sync.dma_start(gi[:], g_ext[eb])
                gb = gpool.tile([128, bw * 128], F32, tag="gb", name="gb")
                for j in range(bw):
                    nc.gpsimd.indirect_dma_start(
                        out=gb[:, j * 128:(j + 1) * 128], out_offset=None,
                        in_=tbl[:, :],
                        in_offset=bass.IndirectOffsetOnAxis(
                            ap=gi[:, j:j + 1], axis=0))
                sgt = mpool.tile([128, bw], F32, tag="sgt", name="sgt")
                nc.sync.dma_start(sgt[:], sg_ext[eb])
                rct = mpool.tile([128, bw], F32, tag="rct", name="rct")
                nc.sync.dma_start(rct[:], rc_ext[eb])
                mt = None
                for bi in range(BIN_ROUND):
                    b = eb * BIN_ROUND + bi
                    oh = ohpool.tile([128, T * NPB], F32, tag="oh", name="oh")
                    oh3 = oh[:].rearrange("p (t q) -> p t q", q=NPB)
                    nc.vector.tensor_tensor(
                        out=oh3,
                        in0=sgt[:, bi * T:(bi + 1) * T].to_broadcast(
                            [128, T, NPB]),
                        in1=iota_rep(T), op=mybir.AluOpType.is_equal)
                    nc.vector.tensor_tensor(
                        out=oh3, in0=oh3,
                        in1=rct[:, bi * T:(bi + 1) * T].to_broadcast(
                            [128, T, NPB]),
                        op=mybir.AluOpType.mult)
                    ps = pseg.tile([128, NPB], F32, tag="ps", name="ps")
                    for t in range(T):
                        cx = (bi * T + t) * 128
                        nc.tensor.matmul(ps[:], lhsT=gb[:, cx:cx + 128],
                                         rhs=oh[:, t * NPB:(t + 1) * NPB],
                                         start=(t == 0), stop=(t == T - 1))
                    if b % GROUP == 0:
                        mt = stpool.tile([128, GROUP * NPB], F32, tag="mt",
                                         name="mt")
                    qq = (b % GROUP) * NPB
                    nc.vector.tensor_copy(mt[:, qq:qq + NPB], ps[:])
                    if b % GROUP == GROUP - 1:
                        g = b // GROUP
                        wp = pw.tile([128, GROUP * NPB], F32, tag="wp",
                                     name="wp")
                        nc.tensor.matmul(wp[:], lhsT=Wl[:], rhs=mt[:],
                                         start=True, stop=False)
                        nc.tensor.matmul(wp[:], lhsT=Wr[:],
                                         rhs=ownT[:, g * 128:(g + 1) * 128],
                                         start=False, stop=True)
                        if layer_no == 1:
                            hT = stpool.tile([128, 128], F32, tag="hT",
                                             name="hT")
                            nc.scalar.activation(
                                out=hT[:], in_=wp[:],
                                func=mybir.ActivationFunctionType.Relu,
                                bias=bs[:, :1])
                            tp = pt.tile([128, 128], F32, tag="tp",
                                         name="tp2")
                            nc.tensor.transpose(tp[:], hT[:], ident[:])
                            hs = stpool.tile([128, 128], F32, tag="hs",
                                             name="hs")
                            nc.vector.tensor_copy(hs[:], tp[:])
                            nc.sync.dma_start(hb[g * 128:(g + 1) * 128, :],
                                              hs[:])
                        else:
                            # int8-quantize the output tile with a
                            # per-feature scale: wire bytes halve again and
                            # quant error (<= amax/126.5) is ~25x inside the
                            # 2e-2 tolerance
                            osb = stpool.tile([128, GROUP * NPB], F32,
                                              tag="os", name="osb")
                            nc.vector.tensor_scalar_add(osb[:], wp[:],
                                                        bs[:, :1])
                            am = stpool.tile([128, 1], F32, tag="am",
                                             name="am")
                            nc.vector.tensor_reduce(
                                am[:], osb[:], axis=mybir.AxisListType.X,
                                op=mybir.AluOpType.max,
                                apply_absolute_value=True)
                            nc.vector.tensor_scalar_max(am[:], am[:], 1e-20)
                            rq = stpool.tile([128, 1], F32, tag="rq",
                                             name="rq")
                            nc.vector.reciprocal(rq[:], am[:])
                            nc.vector.tensor_scalar_mul(rq[:], rq[:], QMAX)
                            oq = stpool.tile([128, GROUP * NPB], I8,
                                             tag="oq", name="oq")
                            nc.scalar.activation(
                                out=oq[:], in_=osb[:],
                                func=mybir.ActivationFunctionType.Identity,
                                scale=rq[:, :1])
                            nc.sync.dma_start(
                                out_ext[g][:, :GROUP * NPB], oq[:])
                            nc.sync.dma_start(
                                out_ext[g][:, GROUP * NPB:].bitcast(F32),
                                am[:, :1])

        ownT1 = const.tile([128, NSLOT], F32, name="ownT1")
        own_transpose(ownT1, 1)
        layer(1, xfull, g1_ext, W["W1l"], W["W1r"], bias["b1"], ownT1)

        nc.gpsimd.collective_compute(
            "AllGather", mybir.AluOpType.bypass, replica_groups=RG,
            ins=[hb[:, :]], outs=[hfull[:, :]])

        ownT2 = const.tile([128, NSLOT], F32, name="ownT2")
        own_transpose(ownT2, 2)
        layer(2, hfull, g2_ext, W["W2l"], W["W2r"], bias["b2"], ownT2)

    nc.compile()
    return nc


# ---------------------------------------------------------------------------
# host-side runner: cached jit + cached device staging


def _digest(a: np.ndarray) -> tuple:
    a = np.ascontiguousarray(a)
    v = a.view(np.uint8).reshape(-1)
    return (zlib.crc32(v), zlib.adler32(v), a.shape, str(a.dtype))


_MESH = None


def _mesh():
    global _MESH
    if _MESH is None:
        devices = jax.devices()[:NC]
        assert len(devices) == NC
        _MESH = Mesh(np.asarray(devices), ("core",))
    return _MESH


class _Ctx:
    """Compiled program + cached jitted callable for one metadata shape B."""

    def __init__(self, nc: bass.Bass):
        install_neuronx_cc_hook()
        self.nc = nc
        pname = nc.partition_id_tensor.name if nc.partition_id_tensor else None
        in_names, out_names, out_avals = [], [], []
        for alloc in nc.m.functions[0].allocations:
            if not isinstance(alloc, mybir.MemoryLocationSet):
                continue
            name = alloc.memorylocations[0].name
            if alloc.kind == "ExternalInput":
                if name != pname:
                    in_names.append(name)
            elif alloc.kind == "ExternalOutput":
                out_names.append(name)
                out_avals.append(jax.core.ShapedArray(
                    tuple(alloc.tensor_shape), mybir.dt.np(alloc.dtype)))
        self.in_names = in_names
        self.out_names = out_names
        self.out_avals = out_avals
        n_params = len(in_names)
        # the kernel writes every element of its outputs, so no pre-zeroed
        # donated output operands are needed — PJRT's (uninitialized) result
        # allocations are written in full by the NEFF
        all_names = in_names + ([pname] if pname else [])

        def _body(*args):
            operands = list(args)
            if pname:
                operands.append(partition_id_tensor())
            outs = _bass_exec_p.bind(
                *operands, out_avals=tuple(out_avals),
                in_names=tuple(all_names), out_names=tuple(out_names),
                lowering_input_output_aliases=(), sim_require_finite=True,
                sim_require_nnan=True, nc=nc)
            return tuple(outs)

        mesh = _mesh()
        self.sharding = NamedSharding(mesh, PartitionSpec("core"))
        self.sharded = jax.jit(
            shard_map(_body, mesh=mesh,
                      in_specs=(PartitionSpec("core"),) * n_params,
                      out_specs=(PartitionSpec("core"),) * len(out_names),
                      check_rep=False),
            keep_unused=True)

    def run(self, staged: dict):
        args = [staged[n] for n in self.in_names]
        outs = self.sharded(*args)
        return dict(zip(self.out_names, outs))


_PROGRAMS: dict = {}
_STAGED: dict = {}
_MD_CACHE: dict = {}
LAST_EXEC_NS = None


def _stage(name: str, key: str, build):
    """device_put(build()) once per (name, content-key); reuse afterwards."""
    k = (name, key)
    arr = _STAGED.get(k)
    if arr is None:
        arr = jax.device_put(build(), NamedSharding(_mesh(),
                                                    PartitionSpec("core")))
        _STAGED[k] = arr
    return arr


def kernel(**inputs) -> np.ndarray:
    x = np.asarray(inputs["x"], np.float32)
    edge_index = np.asarray(inputs["edge_index"])

    ek = _digest(edge_index)
    md = _MD_CACHE.get(ek)
    if md is None:
        md = _MD_CACHE[ek] = build_metadata(edge_index)
    B = md["B"]

    ctx = _PROGRAMS.get(B)
    if ctx is None:
        ctx = _PROGRAMS[B] = _Ctx(build_program(B))

    # per-core [...] metadata arrays are staged as one global array whose
    # axis-0 shard c is core c's slice (shard_map in_specs=P("core"))
    staged = {
        "xs": _stage("xs", _digest(x), lambda: np.ascontiguousarray(x)),
        "iota": _stage("iota", "static",
                       lambda: np.tile(md["iota"], (NC, 1))),
    }
    for nm in ("g1", "g2", "sg", "rc", "own"):
        a = md[nm]
        staged[nm] = _stage(nm, ek, partial(
            lambda a: np.ascontiguousarray(a.reshape(-1, *a.shape[2:])), a))
    for nm in ("W1l", "W1r", "W2l", "W2r"):
        w = np.asarray(inputs[nm], np.float32)
        staged[nm] = _stage(nm, _digest(w), partial(
            lambda w: np.ascontiguousarray(np.tile(w, (NC, 1))), w))
    for nm in ("b1", "b2"):
        b = np.asarray(inputs[nm], np.float32).reshape(D, 1)
        staged[nm] = _stage(nm, _digest(b), partial(
            lambda b: np.ascontiguousarray(np.tile(b, (NC, 1))), b))

    outs = ctx.run(staged)
    packed = np.asarray(outs["outT"])  # [NC*NGROUP, D, GROUP*NPB+4] i8
    payload = packed[:, :, :GROUP * NPB]
    scale = np.ascontiguousarray(
        packed[:, :, GROUP * NPB:]).view(np.float32) * (1.0 / QMAX)
    # i8 feature-major -> dequantized node-major f32
    deq = payload.astype(np.float32) * scale
    full = deq.transpose(0, 2, 1).reshape(-1, D)
    return np.ascontiguousarray(full[md["node_pos"]])


if __name__ == "__main__":
    import reference
    inputs = {k: np.asarray(v) for k, v in reference.setup_inputs().items()}
    out = kernel(**inputs)
    print(out.shape, out.dtype)


# revision 23
# speedup vs baseline: 1.1619x; 1.1619x over previous
"""Trainium2 Bass kernel for a 2-layer GraphSAGE (segment-mean aggregation).

Single fused SPMD program on 8 cores. Nodes are sharded contiguously by id;
edges partitioned by destination so each core's scatter-mean is local. The
halo exchanges are ON-DEVICE AllGather collectives: x shards are gathered
into a Shared-DRAM table before layer 1, and the layer-1 node features are
gathered into a second Shared table before layer 2 — no host round-trip.

Per bin (<=32 consecutive nodes, <=512 edges) the device gathers table rows
(indirect DMA, one 128-row gather per edge tile), builds a recip-scaled
one-hot on DVE, and a TensorE matmul accumulates feature-major segment means
into PSUM. Per 4 bins, two more matmuls apply W_l/W_r and an epilogue adds
bias (+relu between layers). The final output travels over the wire as fp16
(tolerance is 2e-2; fp16 rounding is ~1e-3 of max).

Host side: the axon tunnel moves ~30 MB/s, so everything re-usable is staged
on device once and cached by content digest — x, gather metadata, weights,
and the jitted shard_map callable. Steady-state per-call traffic is just the
fp16 output fetch.
"""

import sys
import zlib
from contextlib import ExitStack
from functools import partial

import numpy as np

try:
    import concourse.bass as bass
except ImportError:  # pragma: no cover
    sys.path.insert(0, "/opt/trn_rl_repo")
    import concourse.bass as bass

import jax
import jax.numpy as jnp
from jax.sharding import Mesh, NamedSharding, PartitionSpec

from jax.experimental.shard_map import shard_map

import concourse.bacc as bacc
import concourse.mybir as mybir
import concourse.tile as tile
from concourse.bass2jax import (
    _bass_exec_p,
    install_neuronx_cc_hook,
    partition_id_tensor,
)
from concourse.masks import make_identity

N = 50000
E = 800000
D = 128
NC = 8
NSH = N // NC
T = 4
SLOTS_PER_BIN = T * 128
NPB = 32
GROUP = 4
BIN_ROUND = 8
OWN_CB = 4

F32 = mybir.dt.float32
F16 = mybir.dt.float16
I32 = mybir.dt.int32
I8 = mybir.dt.int8
QMAX = 126.5


def build_metadata(edge_index, n_nodes=N, n_cores=NC):
    src = np.asarray(edge_index[0], dtype=np.int64)
    dst = np.asarray(edge_index[1], dtype=np.int64)
    nsh = n_nodes // n_cores
    deg = np.bincount(dst, minlength=n_nodes)
    assert deg.max() <= SLOTS_PER_BIN
    recip = np.zeros(n_nodes, np.float32)
    nz = deg > 0
    recip[nz] = (1.0 / deg[nz]).astype(np.float32)

    order = np.argsort(dst, kind="stable")
    src_s = src[order]
    indptr = np.zeros(n_nodes + 1, np.int64)
    indptr[1:] = np.cumsum(deg)

    core_bins = []
    for c in range(n_cores):
        lo, hi = c * nsh, (c + 1) * nsh
        bins = []
        i = lo
        while i < hi:
            start = i
            s = 0
            while i < hi and (i - start) < NPB and s + deg[i] <= SLOTS_PER_BIN:
                s += deg[i]
                i += 1
            bins.append((start, i - start))
        core_bins.append(bins)

    B = max(len(b) for b in core_bins)
    B = -(-B // BIN_ROUND) * BIN_ROUND
    NSLOT = B * NPB
    OWN_C = NSLOT // 128
    NBATCH = B // BIN_ROUND
    OWN_CHUNKS = -(-OWN_C // OWN_CB)

    C = B * T
    gidx1 = np.zeros((n_cores, 128, C), np.int32)
    gidx2 = np.zeros((n_cores, 128, C), np.int32)
    seg = np.zeros((n_cores, 128, C), np.float32)
    rcp = np.zeros((n_cores, 128, C), np.float32)
    # local node id per slot; pad slots point at the zero row (nsh), which
    # the layer-1 own-gather reads harmlessly and the output scatter uses
    # as the dump row
    ownidx = np.full((n_cores, 128, OWN_C), nsh, np.int32)
    node_pos = np.full(n_nodes, -1, np.int64)

    for c in range(n_cores):
        for b, (nlo, nn) in enumerate(core_bins[c]):
            base = b * NPB
            nodes = np.arange(nlo, nlo + nn)
            slots = base + np.arange(nn)
            node_pos[nodes] = c * NSLOT + slots
            ownidx[c, slots % 128, slots // 128] = nodes - c * nsh
            degs = deg[nodes]
            ne = int(degs.sum())
            if ne == 0:
                continue
            s = np.arange(ne)
            q = np.repeat(np.arange(nn), degs)
            e0 = indptr[nlo]
            t_, p_ = s // 128, s % 128
            col = b * T + t_
            gidx1[c, p_, col] = src_s[e0:e0 + ne]
            seg[c, p_, col] = q
            rcp[c, p_, col] = np.repeat(recip[nodes], degs)

    assert np.all(node_pos >= 0)
    for c in range(n_cores):
        g2 = node_pos[gidx1[c]].astype(np.int32)
        g2[rcp[c] == 0.0] = 0
        gidx2[c] = g2
        # x table rows are per-core blocks of nsh+1 (zero pad row per core):
        # global row of node n is n + n//nsh
        gidx1[c] += gidx1[c] // nsh

    def batched(a, w):
        nb = a.shape[-1] // w
        return np.ascontiguousarray(
            a.reshape(a.shape[0], 128, nb, w).transpose(0, 2, 1, 3))

    bw = BIN_ROUND * T
    md = dict(B=B, C=C, NSLOT=NSLOT, OWN_C=OWN_C, NBATCH=NBATCH,
              OWN_CHUNKS=OWN_CHUNKS, node_pos=node_pos, own=ownidx,
              g1=batched(gidx1, bw), g2=batched(gidx2, bw),
              sg=batched(seg, bw), rc=batched(rcp, bw))
    md["iota"] = np.tile(np.arange(NPB, dtype=np.float32), (128, 1))
    return md


def build_program(B, n_nodes=N, n_cores=NC):
    NSLOT = B * NPB
    OWN_C = NSLOT // 128
    NBATCH = B // BIN_ROUND
    OWN_CHUNKS = -(-OWN_C // OWN_CB)
    NGROUP = B // GROUP
    bw = BIN_ROUND * T
    RG = [list(range(n_cores))]

    nc = bacc.Bacc("TRN2", target_bir_lowering=False, debug=False,
                   num_devices=n_cores)

    # x shard carries a trailing zero row: pad slots gather it, and the
    # output scatter dumps pad rows at the same index (NSH)
    xs_ext = nc.dram_tensor("xs", [NSH + 1, D], F32, kind="ExternalInput")
    g1_ext = nc.dram_tensor("g1", [NBATCH, 128, bw], I32, kind="ExternalInput")
    g2_ext = nc.dram_tensor("g2", [NBATCH, 128, bw], I32, kind="ExternalInput")
    sg_ext = nc.dram_tensor("sg", [NBATCH, 128, bw], F32, kind="ExternalInput")
    rc_ext = nc.dram_tensor("rc", [NBATCH, 128, bw], F32, kind="ExternalInput")
    own_ext = nc.dram_tensor("own", [128, OWN_C], I32, kind="ExternalInput")
    iota_ext = nc.dram_tensor("iota", [128, NPB], F32, kind="ExternalInput")
    w_ext = {k: nc.dram_tensor(k, [D, D], F32, kind="ExternalInput")
             for k in ("W1l", "W1r", "W2l", "W2r")}
    b_ext = {k: nc.dram_tensor(k, [D, 1], F32, kind="ExternalInput")
             for k in ("b1", "b2")}
    # node-major packed output: row n holds 128 int8 payload bytes + a
    # 4-byte f32 per-node scale; row NSH is the pad-slot dump row. A single
    # fetch round-trip and no host-side reorder gather.
    out_ext = nc.dram_tensor("outT", [NSH + 1, D + 4], I8,
                             kind="ExternalOutput")

    xb = nc.dram_tensor("xb", [NSH + 1, D], F32)
    xfull = nc.dram_tensor("xfull", [n_cores * (NSH + 1), D], F32,
                           addr_space="Shared")
    hb = nc.dram_tensor("hb", [NSLOT, D], F32)
    hfull = nc.dram_tensor("hfull", [n_cores * NSLOT, D], F32,
                           addr_space="Shared")

    with tile.TileContext(nc) as tc, ExitStack() as ctx:
        const = ctx.enter_context(tc.tile_pool(name="const", bufs=1))
        gpool = ctx.enter_context(tc.tile_pool(name="gather", bufs=3))
        mpool = ctx.enter_context(tc.tile_pool(name="meta", bufs=4))
        ohpool = ctx.enter_context(tc.tile_pool(name="oh", bufs=4))
        stpool = ctx.enter_context(tc.tile_pool(name="stage", bufs=4))
        pseg = ctx.enter_context(tc.tile_pool(name="pseg", bufs=2, space="PSUM"))
        pw = ctx.enter_context(tc.tile_pool(name="pw", bufs=2, space="PSUM"))
        pt = ctx.enter_context(tc.tile_pool(name="pt", bufs=2, space="PSUM"))

        # kick off the x all-gather first; layer-1 edge gathers wait on it,
        # the own-feature path below does not
        nc.gpsimd.dma_start(xb[:, :], xs_ext[:, :])
        nc.gpsimd.collective_compute(
            "AllGather", mybir.AluOpType.bypass, replica_groups=RG,
            ins=[xb[:, :]], outs=[xfull[:, :]])

        W = {}
        for k in ("W1l", "W1r", "W2l", "W2r"):
            W[k] = const.tile([D, D], F32, name=k)
            nc.sync.dma_start(W[k][:], w_ext[k][:, :])
        bias = {}
        for k in ("b1", "b2"):
            bias[k] = const.tile([D, 1], F32, name=k)
            nc.sync.dma_start(bias[k][:], b_ext[k][:, :])
        iota_sb = const.tile([128, NPB], F32, name="iota_sb")
        nc.sync.dma_start(iota_sb[:], iota_ext[:, :])
        ident = const.tile([128, 128], F32, name="ident")
        make_identity(nc, ident[:])
        own_sb = const.tile([128, OWN_C], I32, name="own_sb")
        nc.sync.dma_start(own_sb[:], own_ext[:, :])

        def iota_rep(k):
            ap = iota_sb[:, :]
            return bass.AP(ap.tensor, ap.offset,
                           [[NPB, 128], [0, k], [1, NPB]])

        def own_transpose(ownT, layer):
            """ownT = (own-node features)^T, feature-major [128, NSLOT]."""
            if layer == 1:
                for chk in range(OWN_CHUNKS):
                    kk = min(OWN_CB, OWN_C - chk * OWN_CB)
                    ob = gpool.tile([128, OWN_CB * 128], F32, tag="ob",
                                    name="ob")
                    for j in range(kk):
                        col = chk * OWN_CB + j
                        nc.gpsimd.indirect_dma_start(
                            out=ob[:, j * 128:(j + 1) * 128], out_offset=None,
                            in_=xs_ext[:, :],
                            in_offset=bass.IndirectOffsetOnAxis(
                                ap=own_sb[:, col:col + 1], axis=0))
                    for j in range(kk):
                        col = chk * OWN_CB + j
                        tp = pt.tile([128, 128], F32, tag="tp", name="tp")
                        nc.tensor.transpose(
                            tp[:], ob[:, j * 128:(j + 1) * 128], ident[:])
                        nc.vector.tensor_copy(
                            ownT[:, col * 128:(col + 1) * 128], tp[:])
            else:
                for g in range(OWN_C):
                    ho = gpool.tile([128, 128], F32, tag="ho", name="ho")
                    nc.sync.dma_start(ho[:], hb[g * 128:(g + 1) * 128, :])
                    tp = pt.tile([128, 128], F32, tag="tp", name="tp")
                    nc.tensor.transpose(tp[:], ho[:], ident[:])
                    nc.vector.tensor_copy(ownT[:, g * 128:(g + 1) * 128],
                                          tp[:])

        def layer(layer_no, tbl, g_ext, Wl, Wr, bs, ownT):
            for eb in range(NBATCH):
                gi = mpool.tile([128, bw], I32, tag="gi", name="gi")
                nc.sync.dma_start(gi[:], g_ext[eb])
                gb = gpool.tile([128, bw * 128], F32, tag="gb", name="gb")
                for j in range(bw):
                    nc.gpsimd.indirect_dma_start(
                        out=gb[:, j * 128:(j + 1) * 128], out_offset=None,
                        in_=tbl[:, :],
                        in_offset=bass.IndirectOffsetOnAxis(
                            ap=gi[:, j:j + 1], axis=0))
                sgt = mpool.tile([128, bw], F32, tag="sgt", name="sgt")
                nc.sync.dma_start(sgt[:], sg_ext[eb])
                rct = mpool.tile([128, bw], F32, tag="rct", name="rct")
                nc.sync.dma_start(rct[:], rc_ext[eb])
                mt = None
                for bi in range(BIN_ROUND):
                    b = eb * BIN_ROUND + bi
                    oh = ohpool.tile([128, T * NPB], F32, tag="oh", name="oh")
                    oh3 = oh[:].rearrange("p (t q) -> p t q", q=NPB)
                    nc.vector.tensor_tensor(
                        out=oh3,
                        in0=sgt[:, bi * T:(bi + 1) * T].to_broadcast(
                            [128, T, NPB]),
                        in1=iota_rep(T), op=mybir.AluOpType.is_equal)
                    nc.vector.tensor_tensor(
                        out=oh3, in0=oh3,
                        in1=rct[:, bi * T:(bi + 1) * T].to_broadcast(
                            [128, T, NPB]),
                        op=mybir.AluOpType.mult)
                    ps = pseg.tile([128, NPB], F32, tag="ps", name="ps")
                    for t in range(T):
                        cx = (bi * T + t) * 128
                        nc.tensor.matmul(ps[:], lhsT=gb[:, cx:cx + 128],
                                         rhs=oh[:, t * NPB:(t + 1) * NPB],
                                         start=(t == 0), stop=(t == T - 1))
                    if b % GROUP == 0:
                        mt = stpool.tile([128, GROUP * NPB], F32, tag="mt",
                                         name="mt")
                    qq = (b % GROUP) * NPB
                    nc.vector.tensor_copy(mt[:, qq:qq + NPB], ps[:])
                    if b % GROUP == GROUP - 1:
                        g = b // GROUP
                        wp = pw.tile([128, GROUP * NPB], F32, tag="wp",
                                     name="wp")
                        nc.tensor.matmul(wp[:], lhsT=Wl[:], rhs=mt[:],
                                         start=True, stop=False)
                        nc.tensor.matmul(wp[:], lhsT=Wr[:],
                                         rhs=ownT[:, g * 128:(g + 1) * 128],
                                         start=False, stop=True)
                        if layer_no == 1:
                            hT = stpool.tile([128, 128], F32, tag="hT",
                                             name="hT")
                            nc.scalar.activation(
                                out=hT[:], in_=wp[:],
                                func=mybir.ActivationFunctionType.Relu,
                                bias=bs[:, :1])
                            tp = pt.tile([128, 128], F32, tag="tp",
                                         name="tp2")
                            nc.tensor.transpose(tp[:], hT[:], ident[:])
                            hs = stpool.tile([128, 128], F32, tag="hs",
                                             name="hs")
                            nc.vector.tensor_copy(hs[:], tp[:])
                            nc.sync.dma_start(hb[g * 128:(g + 1) * 128, :],
                                              hs[:])
                        else:
                            # add bias, transpose to node-major, then
                            # int8-quantize with a per-node scale: quant
                            # error (<= amax/126.5) is ~25x inside the 2e-2
                            # tolerance. The f32 scale is packed into the
                            # row tail and the whole 132-byte row scattered
                            # straight to its node id — no host reorder.
                            osb = stpool.tile([128, GROUP * NPB], F32,
                                              tag="os", name="osb")
                            nc.vector.tensor_scalar_add(osb[:], wp[:],
                                                        bs[:, :1])
                            tp = pt.tile([128, 128], F32, tag="tp",
                                         name="tp3")
                            nc.tensor.transpose(tp[:], osb[:], ident[:])
                            am = stpool.tile([128, 1], F32, tag="am",
                                             name="am")
                            nc.vector.tensor_reduce(
                                am[:], tp[:], axis=mybir.AxisListType.X,
                                op=mybir.AluOpType.max,
                                apply_absolute_value=True)
                            nc.vector.tensor_scalar_max(am[:], am[:], 1e-20)
                            rq = stpool.tile([128, 1], F32, tag="rq",
                                             name="rq")
                            nc.vector.reciprocal(rq[:], am[:])
                            nc.vector.tensor_scalar_mul(rq[:], rq[:], QMAX)
                            oq = stpool.tile([128, D + 4], I8,
                                             tag="oq", name="oq")
                            nc.scalar.activation(
                                out=oq[:, :D], in_=tp[:],
                                func=mybir.ActivationFunctionType.Identity,
                                scale=rq[:, :1])
                            nc.vector.tensor_copy(
                                oq[:, D:D + 4].bitcast(F32), am[:, :1])
                            nc.gpsimd.indirect_dma_start(
                                out=out_ext[:, :],
                                out_offset=bass.IndirectOffsetOnAxis(
                                    ap=own_sb[:, g:g + 1], axis=0),
                                in_=oq[:, :], in_offset=None)

        ownT1 = const.tile([128, NSLOT], F32, name="ownT1")
        own_transpose(ownT1, 1)
        layer(1, xfull, g1_ext, W["W1l"], W["W1r"], bias["b1"], ownT1)

        nc.gpsimd.collective_compute(
            "AllGather", mybir.AluOpType.bypass, replica_groups=RG,
            ins=[hb[:, :]], outs=[hfull[:, :]])

        ownT2 = const.tile([128, NSLOT], F32, name="ownT2")
        own_transpose(ownT2, 2)
        layer(2, hfull, g2_ext, W["W2l"], W["W2r"], bias["b2"], ownT2)

    nc.compile()
    return nc


# ---------------------------------------------------------------------------
# host-side runner: cached jit + cached device staging


def _digest(a: np.ndarray) -> tuple:
    a = np.ascontiguousarray(a)
    v = a.view(np.uint8).reshape(-1)
    head = v[:16].tobytes() + v[-16:].tobytes() if v.size >= 16 else v.tobytes()
    return (zlib.crc32(v), head, a.shape, str(a.dtype))


_MESH = None


def _mesh():
    global _MESH
    if _MESH is None:
        devices = jax.devices()[:NC]
        assert len(devices) == NC
        _MESH = Mesh(np.asarray(devices), ("core",))
    return _MESH


class _Ctx:
    """Compiled program + cached jitted callable for one metadata shape B."""

    def __init__(self, nc: bass.Bass):
        install_neuronx_cc_hook()
        self.nc = nc
        pname = nc.partition_id_tensor.name if nc.partition_id_tensor else None
        in_names, out_names, out_avals = [], [], []
        for alloc in nc.m.functions[0].allocations:
            if not isinstance(alloc, mybir.MemoryLocationSet):
                continue
            name = alloc.memorylocations[0].name
            if alloc.kind == "ExternalInput":
                if name != pname:
                    in_names.append(name)
            elif alloc.kind == "ExternalOutput":
                out_names.append(name)
                out_avals.append(jax.core.ShapedArray(
                    tuple(alloc.tensor_shape), mybir.dt.np(alloc.dtype)))
        self.in_names = in_names
        self.out_names = out_names
        self.out_avals = out_avals
        n_params = len(in_names)
        # the kernel writes every element of its outputs, so no pre-zeroed
        # donated output operands are needed — PJRT's (uninitialized) result
        # allocations are written in full by the NEFF
        all_names = in_names + ([pname] if pname else [])

        def _body(*args):
            operands = list(args)
            if pname:
                operands.append(partition_id_tensor())
            outs = _bass_exec_p.bind(
                *operands, out_avals=tuple(out_avals),
                in_names=tuple(all_names), out_names=tuple(out_names),
                lowering_input_output_aliases=(), sim_require_finite=True,
                sim_require_nnan=True, nc=nc)
            return tuple(outs)

        mesh = _mesh()
        self.sharding = NamedSharding(mesh, PartitionSpec("core"))
        self.sharded = jax.jit(
            shard_map(_body, mesh=mesh,
                      in_specs=(PartitionSpec("core"),) * n_params,
                      out_specs=(PartitionSpec("core"),) * len(out_names),
                      check_rep=False),
            keep_unused=True)

    def run(self, staged: dict):
        args = [staged[n] for n in self.in_names]
        outs = self.sharded(*args)
        return dict(zip(self.out_names, outs))


_PROGRAMS: dict = {}
_STAGED: dict = {}
_MD_CACHE: dict = {}
LAST_EXEC_NS = None


def _stage(name: str, key, build):
    """device_put(build()) once per (name, content-key); reuse afterwards."""
    k = (name, key)
    arr = _STAGED.get(k)
    if arr is None:
        for stale in [sk for sk in _STAGED if sk[0] == name]:
            del _STAGED[stale]
        arr = jax.device_put(build(), NamedSharding(_mesh(),
                                                    PartitionSpec("core")))
        _STAGED[k] = arr
    return arr


def kernel(**inputs) -> np.ndarray:
    x = np.asarray(inputs["x"], np.float32)
    edge_index = np.asarray(inputs["edge_index"])

    ek = _digest(edge_index)
    md = _MD_CACHE.get(ek)
    if md is None:
        md = _MD_CACHE[ek] = build_metadata(edge_index)
    B = md["B"]

    ctx = _PROGRAMS.get(B)
    if ctx is None:
        ctx = _PROGRAMS[B] = _Ctx(build_program(B))

    def _padded_x():
        xp = np.zeros((NC, NSH + 1, D), np.float32)
        xp[:, :NSH] = x.reshape(NC, NSH, D)
        return xp.reshape(-1, D)

    # per-core [...] metadata arrays are staged as one global array whose
    # axis-0 shard c is core c's slice (shard_map in_specs=P("core"))
    staged = {
        "xs": _stage("xs", _digest(x), _padded_x),
        "iota": _stage("iota", "static",
                       lambda: np.tile(md["iota"], (NC, 1))),
    }
    for nm in ("g1", "g2", "sg", "rc", "own"):
        a = md[nm]
        staged[nm] = _stage(nm, ek, partial(
            lambda a: np.ascontiguousarray(a.reshape(-1, *a.shape[2:])), a))
    for nm in ("W1l", "W1r", "W2l", "W2r"):
        w = np.asarray(inputs[nm], np.float32)
        staged[nm] = _stage(nm, _digest(w), partial(
            lambda w: np.ascontiguousarray(np.tile(w, (NC, 1))), w))
    for nm in ("b1", "b2"):
        b = np.asarray(inputs[nm], np.float32).reshape(D, 1)
        staged[nm] = _stage(nm, _digest(b), partial(
            lambda b: np.ascontiguousarray(np.tile(b, (NC, 1))), b))

    outs = ctx.run(staged)
    packed = np.asarray(outs["outT"])  # [NC*(NSH+1), D+4] i8, node-major
    pk = packed.reshape(NC, NSH + 1, D + 4)[:, :NSH].reshape(N, D + 4)
    scale = np.ascontiguousarray(pk[:, D:]).view(np.float32) * (1.0 / QMAX)
    return pk[:, :D] * scale


if __name__ == "__main__":
    import reference
    inputs = {k: np.asarray(v) for k, v in reference.setup_inputs().items()}
    out = kernel(**inputs)
    print(out.shape, out.dtype)


# revision 26
# speedup vs baseline: 1.3202x; 1.1362x over previous
"""Trainium2 Bass kernel for a 2-layer GraphSAGE (segment-mean aggregation).

Single fused SPMD program on 8 cores. Nodes are sharded contiguously by id;
edges partitioned by destination so each core's scatter-mean is local. The
halo exchanges are ON-DEVICE AllGather collectives: x shards are gathered
into a Shared-DRAM table before layer 1, and the layer-1 node features are
gathered into a second Shared table before layer 2 — no host round-trip.

Per bin (<=32 consecutive nodes, <=512 edges) the device gathers table rows
(indirect DMA, one 128-row gather per edge tile), builds a recip-scaled
one-hot on DVE, and a TensorE matmul accumulates feature-major segment means
into PSUM. Per 4 bins, two more matmuls apply W_l/W_r and an epilogue adds
bias (+relu between layers). The layer-2 epilogue transposes to node-major,
int8-quantizes each node row with its own f32 scale (tolerance is 2e-2;
quant error is ~4e-3 of max), packs payload+scale into one 132-byte row, and
scatters it by node id via indirect DMA — the fetched buffer IS the output
modulo dequantization.

Host side: the axon tunnel costs ~80-100 ms per RPC plus ~55 MB/s, so wall
time is transfer-dominated: everything reusable is staged on device once and
cached by content digest (x, gather metadata, weights, the jitted shard_map
callable), no donated zero output buffers (the NEFF writes every output
element), and the only steady-state traffic is one exec dispatch plus one
6.6 MB packed-int8 fetch.
"""

import sys
import zlib
from contextlib import ExitStack
from functools import partial

import numpy as np

try:
    import concourse.bass as bass
except ImportError:  # pragma: no cover
    sys.path.insert(0, "/opt/trn_rl_repo")
    import concourse.bass as bass

import jax
from jax.sharding import Mesh, NamedSharding, PartitionSpec
from jax.experimental.shard_map import shard_map

import concourse.bacc as bacc
import concourse.mybir as mybir
import concourse.tile as tile
from concourse.bass2jax import (
    _bass_exec_p,
    install_neuronx_cc_hook,
    partition_id_tensor,
)
from concourse.masks import make_identity

N = 50000
E = 800000
D = 128
NC = 8
NSH = N // NC
T = 4
SLOTS_PER_BIN = T * 128
NPB = 32
GROUP = 4
BIN_ROUND = 8
OWN_CB = 4

F32 = mybir.dt.float32
I32 = mybir.dt.int32
I8 = mybir.dt.int8
QMAX = 126.5


def build_metadata(edge_index, n_nodes=N, n_cores=NC):
    src = np.asarray(edge_index[0], dtype=np.int64)
    dst = np.asarray(edge_index[1], dtype=np.int64)
    nsh = n_nodes // n_cores
    deg = np.bincount(dst, minlength=n_nodes)
    assert deg.max() <= SLOTS_PER_BIN
    recip = np.zeros(n_nodes, np.float32)
    nz = deg > 0
    recip[nz] = (1.0 / deg[nz]).astype(np.float32)

    order = np.argsort(dst, kind="stable")
    src_s = src[order]
    indptr = np.zeros(n_nodes + 1, np.int64)
    indptr[1:] = np.cumsum(deg)

    core_bins = []
    for c in range(n_cores):
        lo, hi = c * nsh, (c + 1) * nsh
        bins = []
        i = lo
        while i < hi:
            start = i
            s = 0
            while i < hi and (i - start) < NPB and s + deg[i] <= SLOTS_PER_BIN:
                s += deg[i]
                i += 1
            bins.append((start, i - start))
        core_bins.append(bins)

    B = max(len(b) for b in core_bins)
    B = -(-B // BIN_ROUND) * BIN_ROUND
    NSLOT = B * NPB
    OWN_C = NSLOT // 128
    NBATCH = B // BIN_ROUND
    OWN_CHUNKS = -(-OWN_C // OWN_CB)

    C = B * T
    gidx1 = np.zeros((n_cores, 128, C), np.int32)
    gidx2 = np.zeros((n_cores, 128, C), np.int32)
    seg = np.zeros((n_cores, 128, C), np.float32)
    rcp = np.zeros((n_cores, 128, C), np.float32)
    # local node id per slot; pad slots point at the zero row (nsh), which
    # the layer-1 own-gather reads harmlessly and the output scatter uses
    # as the dump row
    ownidx = np.full((n_cores, 128, OWN_C), nsh, np.int32)
    node_pos = np.full(n_nodes, -1, np.int64)

    for c in range(n_cores):
        for b, (nlo, nn) in enumerate(core_bins[c]):
            base = b * NPB
            nodes = np.arange(nlo, nlo + nn)
            slots = base + np.arange(nn)
            node_pos[nodes] = c * NSLOT + slots
            ownidx[c, slots % 128, slots // 128] = nodes - c * nsh
            degs = deg[nodes]
            ne = int(degs.sum())
            if ne == 0:
                continue
            s = np.arange(ne)
            q = np.repeat(np.arange(nn), degs)
            e0 = indptr[nlo]
            t_, p_ = s // 128, s % 128
            col = b * T + t_
            gidx1[c, p_, col] = src_s[e0:e0 + ne]
            seg[c, p_, col] = q
            rcp[c, p_, col] = np.repeat(recip[nodes], degs)

    assert np.all(node_pos >= 0)
    for c in range(n_cores):
        g2 = node_pos[gidx1[c]].astype(np.int32)
        g2[rcp[c] == 0.0] = 0
        gidx2[c] = g2
        # x table rows are per-core blocks of nsh+1 (zero pad row per core):
        # global row of node n is n + n//nsh
        gidx1[c] += gidx1[c] // nsh

    def batched(a, w):
        nb = a.shape[-1] // w
        return np.ascontiguousarray(
            a.reshape(a.shape[0], 128, nb, w).transpose(0, 2, 1, 3))

    bw = BIN_ROUND * T
    md = dict(B=B, C=C, NSLOT=NSLOT, OWN_C=OWN_C, NBATCH=NBATCH,
              OWN_CHUNKS=OWN_CHUNKS, node_pos=node_pos, own=ownidx,
              g1=batched(gidx1, bw), g2=batched(gidx2, bw),
              sg=batched(seg, bw), rc=batched(rcp, bw))
    md["iota"] = np.tile(np.arange(NPB, dtype=np.float32), (128, 1))
    return md


def build_program(B, n_nodes=N, n_cores=NC):
    NSLOT = B * NPB
    OWN_C = NSLOT // 128
    NBATCH = B // BIN_ROUND
    OWN_CHUNKS = -(-OWN_C // OWN_CB)
    NGROUP = B // GROUP
    bw = BIN_ROUND * T
    RG = [list(range(n_cores))]

    nc = bacc.Bacc("TRN2", target_bir_lowering=False, debug=False,
                   num_devices=n_cores)

    # x shard carries a trailing zero row: pad slots gather it, and the
    # output scatter dumps pad rows at the same index (NSH)
    xs_ext = nc.dram_tensor("xs", [NSH + 1, D], F32, kind="ExternalInput")
    g1_ext = nc.dram_tensor("g1", [NBATCH, 128, bw], I32, kind="ExternalInput")
    g2_ext = nc.dram_tensor("g2", [NBATCH, 128, bw], I32, kind="ExternalInput")
    sg_ext = nc.dram_tensor("sg", [NBATCH, 128, bw], F32, kind="ExternalInput")
    rc_ext = nc.dram_tensor("rc", [NBATCH, 128, bw], F32, kind="ExternalInput")
    own_ext = nc.dram_tensor("own", [128, OWN_C], I32, kind="ExternalInput")
    iota_ext = nc.dram_tensor("iota", [128, NPB], F32, kind="ExternalInput")
    w_ext = {k: nc.dram_tensor(k, [D, D], F32, kind="ExternalInput")
             for k in ("W1l", "W1r", "W2l", "W2r")}
    b_ext = {k: nc.dram_tensor(k, [D, 1], F32, kind="ExternalInput")
             for k in ("b1", "b2")}
    # node-major packed output: row n holds 128 int8 payload bytes + a
    # 4-byte f32 per-node scale; row NSH is the pad-slot dump row. A single
    # fetch round-trip and no host-side reorder gather.
    out_ext = nc.dram_tensor("outT", [NSH + 1, D + 4], I8,
                             kind="ExternalOutput")

    xb = nc.dram_tensor("xb", [NSH + 1, D], F32)
    xfull = nc.dram_tensor("xfull", [n_cores * (NSH + 1), D], F32,
                           addr_space="Shared")
    hb = nc.dram_tensor("hb", [NSLOT, D], F32)
    hfull = nc.dram_tensor("hfull", [n_cores * NSLOT, D], F32,
                           addr_space="Shared")

    with tile.TileContext(nc) as tc, ExitStack() as ctx:
        const = ctx.enter_context(tc.tile_pool(name="const", bufs=1))
        gpool = ctx.enter_context(tc.tile_pool(name="gather", bufs=3))
        mpool = ctx.enter_context(tc.tile_pool(name="meta", bufs=4))
        ohpool = ctx.enter_context(tc.tile_pool(name="oh", bufs=4))
        stpool = ctx.enter_context(tc.tile_pool(name="stage", bufs=4))
        pseg = ctx.enter_context(tc.tile_pool(name="pseg", bufs=2, space="PSUM"))
        pw = ctx.enter_context(tc.tile_pool(name="pw", bufs=2, space="PSUM"))
        pt = ctx.enter_context(tc.tile_pool(name="pt", bufs=2, space="PSUM"))

        # kick off the x all-gather first; layer-1 edge gathers wait on it,
        # the own-feature path below does not
        nc.gpsimd.dma_start(xb[:, :], xs_ext[:, :])
        nc.gpsimd.collective_compute(
            "AllGather", mybir.AluOpType.bypass, replica_groups=RG,
            ins=[xb[:, :]], outs=[xfull[:, :]])

        W = {}
        for k in ("W1l", "W1r", "W2l", "W2r"):
            W[k] = const.tile([D, D], F32, name=k)
            nc.sync.dma_start(W[k][:], w_ext[k][:, :])
        bias = {}
        for k in ("b1", "b2"):
            bias[k] = const.tile([D, 1], F32, name=k)
            nc.sync.dma_start(bias[k][:], b_ext[k][:, :])
        iota_sb = const.tile([128, NPB], F32, name="iota_sb")
        nc.sync.dma_start(iota_sb[:], iota_ext[:, :])
        ident = const.tile([128, 128], F32, name="ident")
        make_identity(nc, ident[:])
        own_sb = const.tile([128, OWN_C], I32, name="own_sb")
        nc.sync.dma_start(own_sb[:], own_ext[:, :])

        def iota_rep(k):
            ap = iota_sb[:, :]
            return bass.AP(ap.tensor, ap.offset,
                           [[NPB, 128], [0, k], [1, NPB]])

        def own_transpose(ownT, layer):
            """ownT = (own-node features)^T, feature-major [128, NSLOT]."""
            if layer == 1:
                for chk in range(OWN_CHUNKS):
                    kk = min(OWN_CB, OWN_C - chk * OWN_CB)
                    ob = gpool.tile([128, OWN_CB * 128], F32, tag="ob",
                                    name="ob")
                    for j in range(kk):
                        col = chk * OWN_CB + j
                        nc.gpsimd.indirect_dma_start(
                            out=ob[:, j * 128:(j + 1) * 128], out_offset=None,
                            in_=xs_ext[:, :],
                            in_offset=bass.IndirectOffsetOnAxis(
                                ap=own_sb[:, col:col + 1], axis=0))
                    for j in range(kk):
                        col = chk * OWN_CB + j
                        tp = pt.tile([128, 128], F32, tag="tp", name="tp")
                        nc.tensor.transpose(
                            tp[:], ob[:, j * 128:(j + 1) * 128], ident[:])
                        nc.vector.tensor_copy(
                            ownT[:, col * 128:(col + 1) * 128], tp[:])
            else:
                for g in range(OWN_C):
                    ho = gpool.tile([128, 128], F32, tag="ho", name="ho")
                    nc.sync.dma_start(ho[:], hb[g * 128:(g + 1) * 128, :])
                    tp = pt.tile([128, 128], F32, tag="tp", name="tp")
                    nc.tensor.transpose(tp[:], ho[:], ident[:])
                    nc.vector.tensor_copy(ownT[:, g * 128:(g + 1) * 128],
                                          tp[:])

        def layer(layer_no, tbl, g_ext, Wl, Wr, bs, ownT):
            for eb in range(NBATCH):
                gi = mpool.tile([128, bw], I32, tag="gi", name="gi")
                nc.sync.dma_start(gi[:], g_ext[eb])
                gb = gpool.tile([128, bw * 128], F32, tag="gb", name="gb")
                for j in range(bw):
                    nc.gpsimd.indirect_dma_start(
                        out=gb[:, j * 128:(j + 1) * 128], out_offset=None,
                        in_=tbl[:, :],
                        in_offset=bass.IndirectOffsetOnAxis(
                            ap=gi[:, j:j + 1], axis=0))
                sgt = mpool.tile([128, bw], F32, tag="sgt", name="sgt")
                nc.sync.dma_start(sgt[:], sg_ext[eb])
                rct = mpool.tile([128, bw], F32, tag="rct", name="rct")
                nc.sync.dma_start(rct[:], rc_ext[eb])
                mt = None
                for bi in range(BIN_ROUND):
                    b = eb * BIN_ROUND + bi
                    oh = ohpool.tile([128, T * NPB], F32, tag="oh", name="oh")
                    oh3 = oh[:].rearrange("p (t q) -> p t q", q=NPB)
                    nc.vector.tensor_tensor(
                        out=oh3,
                        in0=sgt[:, bi * T:(bi + 1) * T].to_broadcast(
                            [128, T, NPB]),
                        in1=iota_rep(T), op=mybir.AluOpType.is_equal)
                    nc.vector.tensor_tensor(
                        out=oh3, in0=oh3,
                        in1=rct[:, bi * T:(bi + 1) * T].to_broadcast(
                            [128, T, NPB]),
                        op=mybir.AluOpType.mult)
                    ps = pseg.tile([128, NPB], F32, tag="ps", name="ps")
                    for t in range(T):
                        cx = (bi * T + t) * 128
                        nc.tensor.matmul(ps[:], lhsT=gb[:, cx:cx + 128],
                                         rhs=oh[:, t * NPB:(t + 1) * NPB],
                                         start=(t == 0), stop=(t == T - 1))
                    if b % GROUP == 0:
                        mt = stpool.tile([128, GROUP * NPB], F32, tag="mt",
                                         name="mt")
                    qq = (b % GROUP) * NPB
                    nc.vector.tensor_copy(mt[:, qq:qq + NPB], ps[:])
                    if b % GROUP == GROUP - 1:
                        g = b // GROUP
                        wp = pw.tile([128, GROUP * NPB], F32, tag="wp",
                                     name="wp")
                        nc.tensor.matmul(wp[:], lhsT=Wl[:], rhs=mt[:],
                                         start=True, stop=False)
                        nc.tensor.matmul(wp[:], lhsT=Wr[:],
                                         rhs=ownT[:, g * 128:(g + 1) * 128],
                                         start=False, stop=True)
                        if layer_no == 1:
                            hT = stpool.tile([128, 128], F32, tag="hT",
                                             name="hT")
                            nc.scalar.activation(
                                out=hT[:], in_=wp[:],
                                func=mybir.ActivationFunctionType.Relu,
                                bias=bs[:, :1])
                            tp = pt.tile([128, 128], F32, tag="tp",
                                         name="tp2")
                            nc.tensor.transpose(tp[:], hT[:], ident[:])
                            hs = stpool.tile([128, 128], F32, tag="hs",
                                             name="hs")
                            nc.vector.tensor_copy(hs[:], tp[:])
                            nc.sync.dma_start(hb[g * 128:(g + 1) * 128, :],
                                              hs[:])
                        else:
                            # add bias, transpose to node-major, then
                            # int8-quantize with a per-node scale: quant
                            # error (<= amax/126.5) is ~25x inside the 2e-2
                            # tolerance. The f32 scale is packed into the
                            # row tail and the whole 132-byte row scattered
                            # straight to its node id — no host reorder.
                            osb = stpool.tile([128, GROUP * NPB], F32,
                                              tag="os", name="osb")
                            nc.vector.tensor_scalar_add(osb[:], wp[:],
                                                        bs[:, :1])
                            tp = pt.tile([128, 128], F32, tag="tp",
                                         name="tp3")
                            nc.tensor.transpose(tp[:], osb[:], ident[:])
                            am = stpool.tile([128, 1], F32, tag="am",
                                             name="am")
                            nc.vector.tensor_reduce(
                                am[:], tp[:], axis=mybir.AxisListType.X,
                                op=mybir.AluOpType.max,
                                apply_absolute_value=True)
                            nc.vector.tensor_scalar_max(am[:], am[:], 1e-20)
                            rq = stpool.tile([128, 1], F32, tag="rq",
                                             name="rq")
                            nc.vector.reciprocal(rq[:], am[:])
                            nc.vector.tensor_scalar_mul(rq[:], rq[:], QMAX)
                            oq = stpool.tile([128, D + 4], I8,
                                             tag="oq", name="oq")
                            nc.scalar.activation(
                                out=oq[:, :D], in_=tp[:],
                                func=mybir.ActivationFunctionType.Identity,
                                scale=rq[:, :1])
                            nc.vector.tensor_copy(
                                oq[:, D:D + 4].bitcast(F32), am[:, :1])
                            nc.gpsimd.indirect_dma_start(
                                out=out_ext[:, :],
                                out_offset=bass.IndirectOffsetOnAxis(
                                    ap=own_sb[:, g:g + 1], axis=0),
                                in_=oq[:, :], in_offset=None)

        ownT1 = const.tile([128, NSLOT], F32, name="ownT1")
        own_transpose(ownT1, 1)
        layer(1, xfull, g1_ext, W["W1l"], W["W1r"], bias["b1"], ownT1)

        nc.gpsimd.collective_compute(
            "AllGather", mybir.AluOpType.bypass, replica_groups=RG,
            ins=[hb[:, :]], outs=[hfull[:, :]])

        ownT2 = const.tile([128, NSLOT], F32, name="ownT2")
        own_transpose(ownT2, 2)
        layer(2, hfull, g2_ext, W["W2l"], W["W2r"], bias["b2"], ownT2)

    nc.compile()
    return nc


# ---------------------------------------------------------------------------
# host-side runner: cached jit + cached device staging


def _digest(a: np.ndarray) -> tuple:
    a = np.ascontiguousarray(a)
    v = a.view(np.uint8).reshape(-1)
    head = v[:16].tobytes() + v[-16:].tobytes() if v.size >= 16 else v.tobytes()
    return (zlib.crc32(v), head, a.shape, str(a.dtype))


_MESH = None


def _mesh():
    global _MESH
    if _MESH is None:
        devices = jax.devices()[:NC]
        assert len(devices) == NC
        _MESH = Mesh(np.asarray(devices), ("core",))
    return _MESH


class _Ctx:
    """Compiled program + cached jitted callable for one metadata shape B."""

    def __init__(self, nc: bass.Bass):
        install_neuronx_cc_hook()
        self.nc = nc
        pname = nc.partition_id_tensor.name if nc.partition_id_tensor else None
        in_names, out_names, out_avals = [], [], []
        for alloc in nc.m.functions[0].allocations:
            if not isinstance(alloc, mybir.MemoryLocationSet):
                continue
            name = alloc.memorylocations[0].name
            if alloc.kind == "ExternalInput":
                if name != pname:
                    in_names.append(name)
            elif alloc.kind == "ExternalOutput":
                out_names.append(name)
                out_avals.append(jax.core.ShapedArray(
                    tuple(alloc.tensor_shape), mybir.dt.np(alloc.dtype)))
        self.in_names = in_names
        self.out_names = out_names
        self.out_avals = out_avals
        n_params = len(in_names)
        # the kernel writes every element of its outputs, so no pre-zeroed
        # donated output operands are needed — PJRT's (uninitialized) result
        # allocations are written in full by the NEFF
        all_names = in_names + ([pname] if pname else [])

        def _body(*args):
            operands = list(args)
            if pname:
                operands.append(partition_id_tensor())
            outs = _bass_exec_p.bind(
                *operands, out_avals=tuple(out_avals),
                in_names=tuple(all_names), out_names=tuple(out_names),
                lowering_input_output_aliases=(), sim_require_finite=True,
                sim_require_nnan=True, nc=nc)
            return tuple(outs)

        mesh = _mesh()
        self.sharding = NamedSharding(mesh, PartitionSpec("core"))
        self.sharded = jax.jit(
            shard_map(_body, mesh=mesh,
                      in_specs=(PartitionSpec("core"),) * n_params,
                      out_specs=(PartitionSpec("core"),) * len(out_names),
                      check_rep=False),
            keep_unused=True)

    def run(self, staged: dict):
        args = [staged[n] for n in self.in_names]
        outs = self.sharded(*args)
        return dict(zip(self.out_names, outs))


_PROGRAMS: dict = {}
_STAGED: dict = {}
_MD_CACHE: dict = {}
LAST_EXEC_NS = None


def _stage(name: str, key, build):
    """device_put(build()) once per (name, content-key); reuse afterwards."""
    k = (name, key)
    arr = _STAGED.get(k)
    if arr is None:
        for stale in [sk for sk in _STAGED if sk[0] == name]:
            del _STAGED[stale]
        arr = jax.device_put(build(), NamedSharding(_mesh(),
                                                    PartitionSpec("core")))
        _STAGED[k] = arr
    return arr


def kernel(**inputs) -> np.ndarray:
    x = np.asarray(inputs["x"], np.float32)
    edge_index = np.asarray(inputs["edge_index"])

    ek = _digest(edge_index)
    md = _MD_CACHE.get(ek)
    if md is None:
        md = _MD_CACHE[ek] = build_metadata(edge_index)
    B = md["B"]

    ctx = _PROGRAMS.get(B)
    if ctx is None:
        ctx = _PROGRAMS[B] = _Ctx(build_program(B))

    def _padded_x():
        xp = np.zeros((NC, NSH + 1, D), np.float32)
        xp[:, :NSH] = x.reshape(NC, NSH, D)
        return xp.reshape(-1, D)

    # per-core [...] metadata arrays are staged as one global array whose
    # axis-0 shard c is core c's slice (shard_map in_specs=P("core"))
    staged = {
        "xs": _stage("xs", _digest(x), _padded_x),
        "iota": _stage("iota", "static",
                       lambda: np.tile(md["iota"], (NC, 1))),
    }
    for nm in ("g1", "g2", "sg", "rc", "own"):
        a = md[nm]
        staged[nm] = _stage(nm, ek, partial(
            lambda a: np.ascontiguousarray(a.reshape(-1, *a.shape[2:])), a))
    for nm in ("W1l", "W1r", "W2l", "W2r"):
        w = np.asarray(inputs[nm], np.float32)
        staged[nm] = _stage(nm, _digest(w), partial(
            lambda w: np.ascontiguousarray(np.tile(w, (NC, 1))), w))
    for nm in ("b1", "b2"):
        b = np.asarray(inputs[nm], np.float32).reshape(D, 1)
        staged[nm] = _stage(nm, _digest(b), partial(
            lambda b: np.ascontiguousarray(np.tile(b, (NC, 1))), b))

    outs = ctx.run(staged)
    packed = np.asarray(outs["outT"])  # [NC*(NSH+1), D+4] i8, node-major
    pk = packed.reshape(NC, NSH + 1, D + 4)[:, :NSH].reshape(N, D + 4)
    scale = np.ascontiguousarray(pk[:, D:]).view(np.float32) * (1.0 / QMAX)
    return pk[:, :D] * scale


if __name__ == "__main__":
    import reference
    inputs = {k: np.asarray(v) for k, v in reference.setup_inputs().items()}
    out = kernel(**inputs)
    print(out.shape, out.dtype)
